# revision 27
# baseline (speedup 1.0000x reference)
"""AssociativeLIF forward scan on 8 Trainium2 NeuronCores.

Data-parallel over batch B=64 -> 8 per core. Per-core on-chip layout
(identical to the v1 kernel):
  b = b_lo*4 + b_hi  (b_lo in {0,1}, b_hi in {0..3})
  neuron d = j*64 + c  (c = cluster id = d % 64, j = d // 64)
  SBUF partition p = b_lo*64 + c   (128 partitions)
  SBUF free      f = b_hi*64 + j   (256 elements)

v2 changes (DVE is the bottleneck engine at ~98% busy):
 - i-state is pre-scaled by (1-beta_m): v_pre = beta_m*v + i~ is a single
   STT (kills the ACT-produced bv tensor and its cross-engine latency).
 - gain/k, beta_s and (1-beta_m) are folded into the matmul weights on the
   host, so the cascade add is a plain TT broadcast-add from PSUM.
 - the spike-subtract + refractory gate + reset (3 STT ops in v1) are ONE
   fused custom DVE op: v = select(q, VRESET, v_pre - th*(v_pre >= th)).
 - th_eff = 512*q + th stays on the idle ACT engine (uniform-th path).
 - a no_sync_barrier after the cascade matmul pins the scheduler to
   S->F->matmul at the head of each step; the tail ops (A1/V/Q) then fill
   the fp32 matmul's LDW-low/high round trip, so A2(t+1) never stalls.
DVE per step: A2 (casc TT), B (v_pre STT), S (is_ge TT), F (reduce),
| barrier | A1 (decay STT for t+1), V (fused custom), Q (s+s_prev TT).
The recurrence cycle S->F->matmul->A2->B (~2.7us incl the 2-pass fp32
LDWEIGHTS+MATMUL) and the 7-op DVE budget are balanced near the floor;
16-bit DVE fast modes are ruled out by chaos (one fp16 rounding anywhere
flips thousands of spikes -> relerr > 2e-2), PE/GpSimd elementwise
offload measured 760/900ns per [128,256] op (probes), and DMA-accumulate
for q measured a 2-4us serial stall on the Pool DGE queue.

All f32 elementwise rounding matches the reference order closely enough
that spikes are bit-exact (verified vs a numpy replica: 0 flips, v relerr
~2e-7; the only deviations are the host-folded weight scales and the PE's
f32 accumulation order in the 64-term cascade matmul).
"""

import numpy as np

_T, _B, _D = 32, 64, 4096
_NC = 64
_K = _D // _NC  # 64 neurons per cluster
_NCORES = 8
_BLOC = _B // _NCORES  # 8
_VRESET = -0.1
_BIG = 512.0  # q*BIG + th dominates any v_pre; exact in f32
_Q_DMA = False  # DMA-accumulate q: measured 2-4us serial stall; keep on DVE


def _sigmoid_f32(x):
    x64 = np.asarray(x, dtype=np.float64)
    return np.asarray(1.0 / (1.0 + np.exp(-x64)), dtype=np.float32)


_V_OP = None


def _get_v_op():
    """Register (once) the fused v-update custom DVE op:
    out = select(in1 != 0, imm2, in0 - s0*(in0 >= s0))."""
    global _V_OP
    if _V_OP is not None:
        return _V_OP
    from concourse.dve_spec import Spec, Src0, Src1, C0, C2, select
    from concourse.dve_ops import DveOp, OPS
    from concourse.dve_table_gen import dve_ver_for

    name = "ALIF_V_FUSED_ANT"
    for op in OPS:
        if op.name == name:
            _V_OP = op
            return op
    g = Src0 >= C0
    spec = Spec(
        body=select(Src1, C2, Src0 - C0 * g),
        reference=lambda in0, in1, s0, s1, imm2: np.where(
            np.asarray(in1, np.float32) != 0.0,
            np.float32(imm2),
            np.asarray(in0, np.float32)
            - np.float32(s0) * (np.asarray(in0, np.float32) >= np.float32(s0)),
        ).astype(np.float32),
    )
    op = DveOp(name, spec, subdim=False, uops_sha={})
    OPS.append(op)
    # the lookup tables are built at module import; register the new op there
    import concourse.dve_ops as dve_ops_mod
    dve_ops_mod.CUSTOM_DVE_SPECS[name] = spec
    dve_ops_mod._SUB_OPCODE_FOR_NAME[name] = (
        dve_ops_mod._CUSTOM_DVE_ROW_BASE + len(OPS) - 1)
    ver = dve_ver_for("TRN2")
    try:
        op.compile(ver)
    except ValueError as e:
        import re

        m = re.search(r'="([0-9a-f]+)"', str(e))
        if not m:
            raise
        op.uops_sha[ver] = m.group(1)
        op.compile(ver)
    _V_OP = op
    return op


def _build(beta_s, beta_m, th_uniform, probe=False):
    """th_uniform: python float for the uniform-threshold fast path,
    or None for the per-neuron threshold path (th input tensor)."""
    import concourse.bacc as bacc
    import concourse.bass as bass
    import concourse.mybir as mybir
    import concourse.tile as tile

    fp32 = mybir.dt.float32
    bf16 = mybir.dt.bfloat16
    Alu = mybir.AluOpType
    Act = mybir.ActivationFunctionType
    v_op = _get_v_op() if th_uniform is not None else None

    nc = bacc.Bacc("TRN2", target_bir_lowering=False, debug=False,
                   num_devices=_NCORES)

    x_dram = nc.dram_tensor("x", [_T, 128, 256], fp32, kind="ExternalInput")
    if th_uniform is None:
        th_dram = nc.dram_tensor("th", [128, 256], fp32, kind="ExternalInput")
    w_dram = nc.dram_tensor("wblk", [128, 128], fp32, kind="ExternalInput")
    if probe:
        id_dram = nc.dram_tensor("ident", [128, 128], fp32,
                                 kind="ExternalInput")
        pr_dram = nc.dram_tensor("probe_out", [128, 256], fp32,
                                 kind="ExternalOutput")
    s_dram = nc.dram_tensor("s_out", [_T, 128, 256], bf16, kind="ExternalOutput")
    v_dram = nc.dram_tensor("v_out", [_T, 128, 256], fp32, kind="ExternalOutput")

    def bcast_j(ap2):
        """[128, 4] AP -> [128, 4, 64] AP with 0-stride j."""
        return bass.AP(tensor=ap2.tensor, offset=ap2.offset,
                       ap=[list(ap2.ap[0]), list(ap2.ap[1]), [0, _K]])

    def red_ap(ap2):
        """[128, 256] AP -> [128, 4, 64] with packed inner j for the reduce."""
        return bass.AP(tensor=ap2.tensor, offset=ap2.offset,
                       ap=[list(ap2.ap[0]), [_K, 4], [1, _K]])

    with tile.TileContext(nc) as tc:
        with (
            tc.tile_pool(name="singles", bufs=1) as singles,
            tc.tile_pool(name="xp", bufs=6) as xp,
            tc.tile_pool(name="zp", bufs=3) as zp,
            tc.tile_pool(name="ip", bufs=3) as ip,
            tc.tile_pool(name="sp", bufs=5) as sp,
            tc.tile_pool(name="vp", bufs=5) as vp,
            tc.tile_pool(name="vprep", bufs=3) as vprep,
            tc.tile_pool(name="qp", bufs=3) as qp,
            tc.tile_pool(name="tp", bufs=3) as tp,
            tc.tile_pool(name="cfp", bufs=3) as cfp,
            tc.tile_pool(name="psp", bufs=2, space="PSUM") as psp,
            tc.tile_pool(name="pspr", bufs=1, space="PSUM") as pspr,
        ):
            # tiny warmup DMA first: the first transfer on a cold DGE queue
            # completes ~4us late; absorb that on a 4B/row dummy so x~(0)
            # (which gates the first compute op) rides a warm engine
            warmd = singles.tile([128, 1], fp32)
            nc.sync.dma_start(out=warmd[:, :], in_=w_dram[:, 0:1])
            # prefetch the first two input slices before the constant loads
            x_tiles = {}
            for t0 in range(2):
                xt = xp.tile([128, 256], fp32, tag="x")
                nc.sync.dma_start(out=xt[:, :], in_=x_dram[t0, :, :])
                x_tiles[t0] = xt
            w_t = singles.tile([128, 128], fp32)
            nc.sync.dma_start(out=w_t[:, :], in_=w_dram[:, :])
            xkeep = None
            if probe:
                xkeep = singles.tile([128, 256], fp32)
                nc.vector.tensor_copy(out=xkeep[:, :], in_=x_tiles[0][:, :])
            zero_t = singles.tile([128, 256], fp32)
            nc.vector.memset(zero_t[:, :], 0.0)
            zero_bf = singles.tile([128, 256], bf16)
            nc.vector.memset(zero_bf[:, :], 0.0)
            th_eff0 = tp.tile([128, 256], fp32, tag="te")
            th0_t = None
            if th_uniform is None:
                th0_t = singles.tile([128, 256], fp32)
                nc.sync.dma_start(out=th0_t[:, :], in_=th_dram[:, :])
                nc.vector.tensor_copy(out=th_eff0[:, :], in_=th0_t[:, :])
            else:
                nc.vector.memset(th_eff0[:, :], float(th_uniform))
            # warm the ACT function table during initial DMAs
            warm = singles.tile([128, 1], fp32)
            nc.vector.memset(warm[:, :], 0.0)
            nc.scalar.activation(out=warm[:, :], in_=warm[:, :],
                                 func=Act.Copy, bias=0.0, scale=1.0)

            th_eff = th_eff0
            q_cur = zero_bf          # q(0) = 0
            s_prev = zero_bf
            v_prev = zero_t
            i_prev = None            # ~i'(t-1)
            z_next = None            # A1 output for step t
            ps_prev = None

            for t in range(_T):
                last = (t == _T - 1)
                # prefetch x~(t+2) first so the Sync queue issues it ahead of
                # this step's s/v output DMAs (A1(t+1) otherwise waits on it)
                if t + 2 < _T:
                    xt = xp.tile([128, 256], fp32, tag="x")
                    nc.sync.dma_start(out=xt[:, :], in_=x_dram[t + 2, :, :])
                    x_tiles[t + 2] = xt
                # A2: ~i' = z + bcast(ps_prev)   (t=0: ~i' aliases x~(0))
                if t == 0:
                    i_cur = x_tiles[0]
                else:
                    i_cur = ip.tile([128, 256], fp32, tag="i")
                    iv = i_cur[:, :].rearrange("p (b j) -> p b j", j=_K)
                    zv = z_next[:, :].rearrange("p (b j) -> p b j", j=_K)
                    nc.vector.tensor_tensor(out=iv, in0=bcast_j(ps_prev[:, :]),
                                            in1=zv, op=Alu.add)

                # B: v_pre = beta_m*v + ~i'
                v_pre = vprep.tile([128, 256], fp32, tag="v_pre")
                nc.vector.scalar_tensor_tensor(
                    out=v_pre[:, :], in0=v_prev[:, :], scalar=float(beta_m),
                    in1=i_cur[:, :], op0=Alu.mult, op1=Alu.add)

                # S: s = (v_pre >= th_eff)
                s = sp.tile([128, 256], bf16, tag="s")
                nc.vector.tensor_tensor(out=s[:, :], in0=v_pre[:, :],
                                        in1=th_eff[:, :], op=Alu.is_ge)
                nc.sync.dma_start(out=s_dram[t, :, :], in_=s[:, :])

                # F: cf[p, b_hi] = sum_j s   (exact integer counts)
                if not last:
                    cf = cfp.tile([128, 4], fp32, tag="cf")
                    nc.vector.tensor_reduce(
                        out=cf[:, :], in_=red_ap(s[:, :]),
                        axis=mybir.AxisListType.X, op=Alu.add,
                        opt_input=False, opt_output=False)

                    # PE: ps = W~^T-blocks @ cf  (all scales folded into W~)
                    ps = psp.tile([128, 4], fp32, tag="ps")
                    nc.tensor.matmul(ps[:, :], w_t[:, :], cf[:, :],
                                     start=True, stop=True)
                    ps_prev = ps

                    # scheduler fence: keep S->F->matmul at the head of the
                    # step (the list scheduler otherwise floats V/A1 before F,
                    # stalling the cascade recurrence). No runtime sems.
                    tc.no_sync_barrier()

                    # A1 for t+1: z = beta_s*~i' + x~(t+1)  (fills PE latency)
                    z_next = zp.tile([128, 256], fp32, tag="z")
                    nc.vector.scalar_tensor_tensor(
                        out=z_next[:, :], in0=i_cur[:, :],
                        scalar=float(beta_s), in1=x_tiles[t + 1][:, :],
                        op0=Alu.mult, op1=Alu.add)

                # V: v = select(q, VRESET, v_pre - th*(v_pre >= th))
                v_new = vp.tile([128, 256], fp32, tag="v")
                if th_uniform is not None:
                    nc.vector._custom_dve(
                        v_op, out=v_new[:, :], in0=v_pre[:, :],
                        in1=q_cur[:, :], s0=float(th_uniform),
                        imm2=float(_VRESET))
                else:
                    st = vprep.tile([128, 256], fp32, tag="st")
                    nc.vector.tensor_tensor(out=st[:, :], in0=s[:, :],
                                            in1=th0_t[:, :], op=Alu.mult)
                    a = vprep.tile([128, 256], fp32, tag="a")
                    nc.vector.tensor_tensor(out=a[:, :], in0=v_pre[:, :],
                                            in1=st[:, :], op=Alu.subtract)
                    b = vprep.tile([128, 256], fp32, tag="b")
                    nc.vector.scalar_tensor_tensor(
                        out=b[:, :], in0=q_cur[:, :], scalar=0.0,
                        in1=a[:, :], op0=Alu.is_equal, op1=Alu.mult)
                    nc.vector.scalar_tensor_tensor(
                        out=v_new[:, :], in0=q_cur[:, :], scalar=_VRESET,
                        in1=b[:, :], op0=Alu.mult, op1=Alu.add)
                nc.sync.dma_start(out=v_dram[t, :, :], in_=v_new[:, :])

                if not last:
                    # Q: q' = s + s_prev  in {0,1}. On GpSimd: ~900ns there
                    # vs 285 on DVE, but GpSimd is idle and q is off the
                    # recurrence cycle; DVE drops to 6 ops/step.
                    qn = qp.tile([128, 256], bf16, tag="q")
                    nc.gpsimd.tensor_tensor(out=qn[:, :], in0=s[:, :],
                                            in1=s_prev[:, :], op=Alu.add)
                    # TH: th_eff' = BIG*q' + th  (ACT engine, uniform path)
                    th_eff_n = tp.tile([128, 256], fp32, tag="te")
                    if th_uniform is not None:
                        nc.scalar.activation(out=th_eff_n[:, :], in_=qn[:, :],
                                             func=Act.Copy,
                                             bias=float(th_uniform),
                                             scale=float(_BIG))
                    else:
                        nc.vector.scalar_tensor_tensor(
                            out=th_eff_n[:, :], in0=qn[:, :],
                            scalar=float(_BIG), in1=th0_t[:, :],
                            op0=Alu.mult, op1=Alu.add)
                    q_cur = qn
                    th_eff = th_eff_n
                    s_prev = s
                v_prev = v_new
                i_prev = i_cur

            if probe:
                # --- timing probes (identifiable by pc; run once, after the
                # scan) ---
                id_t = singles.tile([128, 128], fp32)
                nc.sync.dma_start(out=id_t[:, :], in_=id_dram[:, :])
                # P1: fp32 identity matmul, 256-col (cost of a PE x-add)
                ps_pr = pspr.tile([128, 256], fp32, tag="pp")
                for _ in range(3):
                    nc.tensor.matmul(ps_pr[:, :], id_t[:, :],
                                     xkeep[:, :], start=True, stop=True)
                pr_sb = singles.tile([128, 256], fp32)
                nc.vector.tensor_copy(out=pr_sb[:, :], in_=ps_pr[:, :])
                nc.sync.dma_start(out=pr_dram[:, :], in_=pr_sb[:, :])
                # P2: bf16 matmul same shape
                id_b = singles.tile([128, 128], bf16)
                nc.vector.tensor_copy(out=id_b[:, :], in_=id_t[:, :])
                xb = singles.tile([128, 256], bf16)
                nc.vector.tensor_copy(out=xb[:, :], in_=xkeep[:, :])
                ps_b = pspr.tile([128, 256], fp32, tag="pp")
                for _ in range(2):
                    nc.tensor.matmul(ps_b[:, :], id_b[:, :], xb[:, :],
                                     start=True, stop=True)
                # P3: GpSimd TT add concurrent with DVE STTs (port-conflict
                # probe): no data deps between the two engines' ops
                ga = singles.tile([128, 256], fp32)
                gb = singles.tile([128, 256], fp32)
                nc.vector.tensor_copy(out=ga[:, :], in_=zero_t[:, :])
                nc.vector.tensor_copy(out=gb[:, :], in_=zero_t[:, :])
                da = singles.tile([128, 256], fp32)
                nc.vector.tensor_copy(out=da[:, :], in_=zero_t[:, :])
                for _ in range(3):
                    nc.gpsimd.tensor_tensor(out=ga[:, :], in0=ga[:, :],
                                            in1=gb[:, :], op=Alu.add)
                    nc.vector.scalar_tensor_tensor(
                        out=da[:, :], in0=da[:, :], scalar=1.0,
                        in1=zero_t[:, :], op0=Alu.mult, op1=Alu.add)
                # P4: f32 reduce (dtype effect on reduce)
                cfx = cfp.tile([128, 4], fp32, tag="cfx")
                for _ in range(2):
                    nc.vector.tensor_reduce(
                        out=cfx[:, :], in_=red_ap(da[:, :]),
                        axis=mybir.AxisListType.X, op=Alu.add,
                        opt_input=False, opt_output=False)
                # P5: all-bf16 flat STT (4x-mode probe)
                sb = singles.tile([128, 256], bf16)
                nc.vector.memset(sb[:, :], 0.0)
                for _ in range(2):
                    nc.vector.scalar_tensor_tensor(
                        out=sb[:, :], in0=sb[:, :], scalar=1.0,
                        in1=zero_bf[:, :], op0=Alu.mult, op1=Alu.add)

    nc.compile()
    return nc


def _prep_inputs(current_in, threshold, beta_mem_raw, beta_syn_raw,
                 neighbor_weights, cluster_gain):
    """Host-side param prep + per-core layout transform."""
    f32 = np.float32
    f64 = np.float64
    beta_m = _sigmoid_f32(beta_mem_raw).reshape(())
    beta_s = _sigmoid_f32(beta_syn_raw).reshape(())
    bm1 = f32(1.0) - beta_m
    Wsig = _sigmoid_f32(neighbor_weights)  # (64, 64)

    gain = np.asarray(cluster_gain, dtype=f32)
    # W~[c',c] = Wsig[c,c'] * gain[c] * beta_s * (1-beta_m) / 64 (one rounding)
    colscale = f64(gain) * f64(beta_s) * f64(bm1) / f64(_K)
    wt64 = f64(Wsig.T) * colscale[None, :]
    wblk = np.zeros((128, 128), dtype=f32)
    wblk[0:64, 0:64] = wt64.astype(f32)
    wblk[64:128, 64:128] = wt64.astype(f32)

    th = np.asarray(threshold, dtype=f32)
    uniform_th = float(th.flat[0]) if np.all(th == th.flat[0]) else None
    th_jc = th.reshape(_K, _NC)  # [j, c]
    th_tile = np.ascontiguousarray(
        np.tile(th_jc.T[:, None, :], (2, 4, 1)).reshape(128, 256), dtype=f32)

    x = np.asarray(current_in, dtype=f32) * bm1   # fold (1-beta_m) into x
    per_core_x = []
    for core in range(_NCORES):
        xl = x[:, core * _BLOC:(core + 1) * _BLOC, :]
        xd = xl.reshape(_T, 2, 4, _K, _NC).transpose(0, 1, 4, 2, 3)
        per_core_x.append(np.ascontiguousarray(xd).reshape(_T, 128, 256))

    return (per_core_x, th_tile, wblk, uniform_th,
            float(beta_s), float(beta_m))


def _gather_output(dev_out):
    """(T,128,256) device layout -> (T, 8, 4096) batch-major."""
    a = dev_out.reshape(_T, 2, _NC, 4, _K).transpose(0, 1, 3, 4, 2)
    return np.ascontiguousarray(a).reshape(_T, _BLOC, _D)


def _run(current_in, threshold, beta_mem_raw, beta_syn_raw,
         neighbor_weights, cluster_gain, trace=False, tmpdir=None,
         force_general=False, probe=False):
    from concourse.bass_utils import run_bass_kernel_spmd

    (per_core_x, th_tile, wblk, uniform_th, beta_s, beta_m) = \
        _prep_inputs(current_in, threshold, beta_mem_raw, beta_syn_raw,
                     neighbor_weights, cluster_gain)

    if force_general:
        uniform_th = None
    nc = _build(beta_s, beta_m, uniform_th, probe=probe)
    in_maps = []
    for c in range(_NCORES):
        m = {"x": per_core_x[c], "wblk": wblk}
        if uniform_th is None:
            m["th"] = th_tile
        if probe:
            m["ident"] = np.eye(128, dtype=np.float32)
        in_maps.append(m)

    res = run_bass_kernel_spmd(nc, in_maps, list(range(_NCORES)),
                               trace=trace, tmpdir=tmpdir)

    spikes = np.empty((_T, _B, _D), dtype=np.float32)
    v_trace = np.empty((_T, _B, _D), dtype=np.float32)
    for core in range(_NCORES):
        b0 = core * _BLOC
        spikes[:, b0:b0 + _BLOC, :] = _gather_output(
            np.asarray(res.results[core]["s_out"], dtype=np.float32))
        v_trace[:, b0:b0 + _BLOC, :] = _gather_output(res.results[core]["v_out"])
    return (spikes, v_trace), res


def kernel(current_in, threshold, beta_mem_raw, beta_syn_raw,
           neighbor_weights, cluster_gain):
    (spikes, v_trace), _ = _run(current_in, threshold, beta_mem_raw,
                                beta_syn_raw, neighbor_weights, cluster_gain)
    return spikes, v_trace


# revision 31
# speedup vs baseline: 1.0374x; 1.0374x over previous
"""AssociativeLIF forward scan on 8 Trainium2 NeuronCores.

Data-parallel over batch B=64 -> 8 per core. Per-core on-chip layout
(identical to the v1 kernel):
  b = b_lo*4 + b_hi  (b_lo in {0,1}, b_hi in {0..3})
  neuron d = j*64 + c  (c = cluster id = d % 64, j = d // 64)
  SBUF partition p = b_lo*64 + c   (128 partitions)
  SBUF free      f = b_hi*64 + j   (256 elements)

v2 changes (DVE is the bottleneck engine at ~98% busy):
 - i-state is pre-scaled by (1-beta_m): v_pre = beta_m*v + i~ is a single
   STT (kills the ACT-produced bv tensor and its cross-engine latency).
 - gain/k, beta_s and (1-beta_m) are folded into the matmul weights on the
   host, so the cascade add is a plain TT broadcast-add from PSUM.
 - the spike-subtract + refractory gate + reset (3 STT ops in v1) are ONE
   fused custom DVE op: v = select(q, VRESET, v_pre - th*(v_pre >= th)).
 - th_eff = 512*q + th stays on the idle ACT engine (uniform-th path).
 - a no_sync_barrier after the cascade matmul pins the scheduler to
   S->F->matmul at the head of each step; the tail ops (A1/V/Q) then fill
   the fp32 matmul's LDW-low/high round trip, so A2(t+1) never stalls.
DVE per step: A2 (casc TT), B (v_pre STT), S (is_ge TT), F (reduce),
| barrier | A1 (decay STT for t+1), V (fused custom), Q (s+s_prev TT).
The recurrence cycle S->F->matmul->A2->B (~2.7us incl the 2-pass fp32
LDWEIGHTS+MATMUL) and the 7-op DVE budget are balanced near the floor;
16-bit DVE fast modes are ruled out by chaos (one fp16 rounding anywhere
flips thousands of spikes -> relerr > 2e-2), PE/GpSimd elementwise
offload measured 760/900ns per [128,256] op (probes), and DMA-accumulate
for q measured a 2-4us serial stall on the Pool DGE queue.

All f32 elementwise rounding matches the reference order closely enough
that spikes are bit-exact (verified vs a numpy replica: 0 flips, v relerr
~2e-7; the only deviations are the host-folded weight scales and the PE's
f32 accumulation order in the 64-term cascade matmul).
"""

import numpy as np

_T, _B, _D = 32, 64, 4096
_NC = 64
_K = _D // _NC  # 64 neurons per cluster
_NCORES = 8
_BLOC = _B // _NCORES  # 8
_VRESET = -0.1
_BIG = 512.0  # q*BIG + th dominates any v_pre; exact in f32
_Q_DMA = False  # DMA-accumulate q: measured 2-4us serial stall; keep on DVE


def _sigmoid_f32(x):
    x64 = np.asarray(x, dtype=np.float64)
    return np.asarray(1.0 / (1.0 + np.exp(-x64)), dtype=np.float32)


_V_OP = None


def _get_v_op():
    """Register (once) the fused v-update custom DVE op:
    out = select(in1 != 0, imm2, in0 - s0*(in0 >= s0))."""
    global _V_OP
    if _V_OP is not None:
        return _V_OP
    from concourse.dve_spec import Spec, Src0, Src1, C0, C2, select
    from concourse.dve_ops import DveOp, OPS
    from concourse.dve_table_gen import dve_ver_for

    name = "ALIF_V_FUSED_ANT"
    for op in OPS:
        if op.name == name:
            _V_OP = op
            return op
    g = Src0 >= C0
    spec = Spec(
        body=select(Src1, C2, Src0 - C0 * g),
        reference=lambda in0, in1, s0, s1, imm2: np.where(
            np.asarray(in1, np.float32) != 0.0,
            np.float32(imm2),
            np.asarray(in0, np.float32)
            - np.float32(s0) * (np.asarray(in0, np.float32) >= np.float32(s0)),
        ).astype(np.float32),
    )
    op = DveOp(name, spec, subdim=False, uops_sha={})
    OPS.append(op)
    # the lookup tables are built at module import; register the new op there
    import concourse.dve_ops as dve_ops_mod
    dve_ops_mod.CUSTOM_DVE_SPECS[name] = spec
    dve_ops_mod._SUB_OPCODE_FOR_NAME[name] = (
        dve_ops_mod._CUSTOM_DVE_ROW_BASE + len(OPS) - 1)
    ver = dve_ver_for("TRN2")
    try:
        op.compile(ver)
    except ValueError as e:
        import re

        m = re.search(r'="([0-9a-f]+)"', str(e))
        if not m:
            raise
        op.uops_sha[ver] = m.group(1)
        op.compile(ver)
    _V_OP = op
    return op


def _build(beta_s, beta_m, th_uniform, probe=False):
    """th_uniform: python float for the uniform-threshold fast path,
    or None for the per-neuron threshold path (th input tensor)."""
    import concourse.bacc as bacc
    import concourse.bass as bass
    import concourse.mybir as mybir
    import concourse.tile as tile

    fp32 = mybir.dt.float32
    bf16 = mybir.dt.bfloat16
    Alu = mybir.AluOpType
    Act = mybir.ActivationFunctionType
    v_op = _get_v_op() if th_uniform is not None else None

    nc = bacc.Bacc("TRN2", target_bir_lowering=False, debug=False,
                   num_devices=_NCORES)

    x_dram = nc.dram_tensor("x", [_T, 128, 256], fp32, kind="ExternalInput")
    if th_uniform is None:
        th_dram = nc.dram_tensor("th", [128, 256], fp32, kind="ExternalInput")
    w_dram = nc.dram_tensor("wblk", [128, 128], fp32, kind="ExternalInput")
    if probe:
        id_dram = nc.dram_tensor("ident", [128, 128], fp32,
                                 kind="ExternalInput")
        pr_dram = nc.dram_tensor("probe_out", [128, 256], fp32,
                                 kind="ExternalOutput")
    s_dram = nc.dram_tensor("s_out", [_T, 128, 256], bf16, kind="ExternalOutput")
    v_dram = nc.dram_tensor("v_out", [_T, 128, 256], fp32, kind="ExternalOutput")

    def bcast_j(ap2):
        """[128, 4] AP -> [128, 4, 64] AP with 0-stride j."""
        return bass.AP(tensor=ap2.tensor, offset=ap2.offset,
                       ap=[list(ap2.ap[0]), list(ap2.ap[1]), [0, _K]])

    def red_ap(ap2):
        """[128, 256] AP -> [128, 4, 64] with packed inner j for the reduce."""
        return bass.AP(tensor=ap2.tensor, offset=ap2.offset,
                       ap=[list(ap2.ap[0]), [_K, 4], [1, _K]])

    with tile.TileContext(nc) as tc:
        with (
            tc.tile_pool(name="singles", bufs=1) as singles,
            tc.tile_pool(name="xp", bufs=6) as xp,
            tc.tile_pool(name="zp", bufs=3) as zp,
            tc.tile_pool(name="ip", bufs=3) as ip,
            tc.tile_pool(name="sp", bufs=5) as sp,
            tc.tile_pool(name="vp", bufs=5) as vp,
            tc.tile_pool(name="vprep", bufs=3) as vprep,
            tc.tile_pool(name="qp", bufs=3) as qp,
            tc.tile_pool(name="tp", bufs=3) as tp,
            tc.tile_pool(name="cfp", bufs=3) as cfp,
            tc.tile_pool(name="psp", bufs=2, space="PSUM") as psp,
            tc.tile_pool(name="pspr", bufs=1, space="PSUM") as pspr,
        ):
            # tiny warmup DMA first: the first transfer on a cold DGE queue
            # completes ~4us late; absorb that on a 4B/row dummy so x~(0)
            # (which gates the first compute op) rides a warm engine
            warmd = singles.tile([128, 1], fp32)
            nc.sync.dma_start(out=warmd[:, :], in_=w_dram[:, 0:1])
            warmg = singles.tile([128, 1], fp32)
            nc.gpsimd.dma_start(out=warmg[:, :], in_=w_dram[:, 1:2])
            # prefetch the first two input slices before the constant loads
            x_tiles = {}
            for t0 in range(2):
                xt = xp.tile([128, 256], fp32, tag="x")
                nc.sync.dma_start(out=xt[:, :], in_=x_dram[t0, :, :])
                x_tiles[t0] = xt
            w_t = singles.tile([128, 128], fp32)
            nc.sync.dma_start(out=w_t[:, :], in_=w_dram[:, :])
            xkeep = None
            if probe:
                xkeep = singles.tile([128, 256], fp32)
                nc.vector.tensor_copy(out=xkeep[:, :], in_=x_tiles[0][:, :])
            zero_t = singles.tile([128, 256], fp32)
            nc.vector.memset(zero_t[:, :], 0.0)
            zero_bf = singles.tile([128, 256], bf16)
            nc.vector.memset(zero_bf[:, :], 0.0)
            th_eff0 = tp.tile([128, 256], fp32, tag="te")
            th0_t = None
            if th_uniform is None:
                th0_t = singles.tile([128, 256], fp32)
                nc.sync.dma_start(out=th0_t[:, :], in_=th_dram[:, :])
                nc.vector.tensor_copy(out=th_eff0[:, :], in_=th0_t[:, :])
            else:
                nc.vector.memset(th_eff0[:, :], float(th_uniform))
            # warm the ACT function table during initial DMAs
            warm = singles.tile([128, 1], fp32)
            nc.vector.memset(warm[:, :], 0.0)
            nc.scalar.activation(out=warm[:, :], in_=warm[:, :],
                                 func=Act.Copy, bias=0.0, scale=1.0)

            th_eff = th_eff0
            q_cur = zero_bf          # q(0) = 0
            s_prev = zero_bf
            v_prev = zero_t
            i_prev = None            # ~i'(t-1)
            z_next = None            # A1 output for step t
            ps_prev = None

            for t in range(_T):
                last = (t == _T - 1)
                # prefetch x~(t+2) first so the Sync queue issues it ahead of
                # this step's s/v output DMAs (A1(t+1) otherwise waits on it)
                if t + 2 < _T:
                    xt = xp.tile([128, 256], fp32, tag="x")
                    nc.sync.dma_start(out=xt[:, :], in_=x_dram[t + 2, :, :])
                    x_tiles[t + 2] = xt
                # A2: ~i' = z + bcast(ps_prev)   (t=0: ~i' aliases x~(0))
                if t == 0:
                    i_cur = x_tiles[0]
                else:
                    i_cur = ip.tile([128, 256], fp32, tag="i")
                    iv = i_cur[:, :].rearrange("p (b j) -> p b j", j=_K)
                    zv = z_next[:, :].rearrange("p (b j) -> p b j", j=_K)
                    nc.vector.tensor_tensor(out=iv, in0=bcast_j(ps_prev[:, :]),
                                            in1=zv, op=Alu.add)

                # B: v_pre = beta_m*v + ~i'   (t=0: v=0, so v_pre IS x~(0))
                if t == 0:
                    v_pre = i_cur
                else:
                    v_pre = vprep.tile([128, 256], fp32, tag="v_pre")
                    nc.vector.scalar_tensor_tensor(
                        out=v_pre[:, :], in0=v_prev[:, :],
                        scalar=float(beta_m), in1=i_cur[:, :],
                        op0=Alu.mult, op1=Alu.add)

                # S: s = (v_pre >= th_eff)
                s = sp.tile([128, 256], bf16, tag="s")
                nc.vector.tensor_tensor(out=s[:, :], in0=v_pre[:, :],
                                        in1=th_eff[:, :], op=Alu.is_ge)
                nc.sync.dma_start(out=s_dram[t, :, :], in_=s[:, :])

                # F: cf[p, b_hi] = sum_j s   (exact integer counts)
                if not last:
                    cf = cfp.tile([128, 4], fp32, tag="cf")
                    nc.vector.tensor_reduce(
                        out=cf[:, :], in_=red_ap(s[:, :]),
                        axis=mybir.AxisListType.X, op=Alu.add,
                        opt_input=False, opt_output=False)

                    # PE: ps = W~^T-blocks @ cf  (all scales folded into W~)
                    ps = psp.tile([128, 4], fp32, tag="ps")
                    nc.tensor.matmul(ps[:, :], w_t[:, :], cf[:, :],
                                     start=True, stop=True)
                    ps_prev = ps

                    # scheduler fence: keep S->F->matmul at the head of the
                    # step (the list scheduler otherwise floats V/A1 before F,
                    # stalling the cascade recurrence). No runtime sems.
                    tc.no_sync_barrier()

                    # A1 for t+1: z = beta_s*~i' + x~(t+1)  (fills PE latency)
                    z_next = zp.tile([128, 256], fp32, tag="z")
                    nc.vector.scalar_tensor_tensor(
                        out=z_next[:, :], in0=i_cur[:, :],
                        scalar=float(beta_s), in1=x_tiles[t + 1][:, :],
                        op0=Alu.mult, op1=Alu.add)

                # V: v = select(q, VRESET, v_pre - th*(v_pre >= th))
                v_new = vp.tile([128, 256], fp32, tag="v")
                if th_uniform is not None:
                    nc.vector._custom_dve(
                        v_op, out=v_new[:, :], in0=v_pre[:, :],
                        in1=q_cur[:, :], s0=float(th_uniform),
                        imm2=float(_VRESET))
                else:
                    st = vprep.tile([128, 256], fp32, tag="st")
                    nc.vector.tensor_tensor(out=st[:, :], in0=s[:, :],
                                            in1=th0_t[:, :], op=Alu.mult)
                    a = vprep.tile([128, 256], fp32, tag="a")
                    nc.vector.tensor_tensor(out=a[:, :], in0=v_pre[:, :],
                                            in1=st[:, :], op=Alu.subtract)
                    b = vprep.tile([128, 256], fp32, tag="b")
                    nc.vector.scalar_tensor_tensor(
                        out=b[:, :], in0=q_cur[:, :], scalar=0.0,
                        in1=a[:, :], op0=Alu.is_equal, op1=Alu.mult)
                    nc.vector.scalar_tensor_tensor(
                        out=v_new[:, :], in0=q_cur[:, :], scalar=_VRESET,
                        in1=b[:, :], op0=Alu.mult, op1=Alu.add)
                # last two v-outs drain on the (pre-warmed) Pool queue in
                # parallel with the Sync queue's s-out, shortening the final
                # DMA-drain barrier; steady-state steps keep the Sync queue
                if t >= _T - 2:
                    nc.gpsimd.dma_start(out=v_dram[t, :, :], in_=v_new[:, :])
                else:
                    nc.sync.dma_start(out=v_dram[t, :, :], in_=v_new[:, :])

                if not last:
                    # Q: q' = s + s_prev  in {0,1} (bf16, 2x DVE mode).
                    # Keep on DVE: GpSimd measured +5us total (port
                    # contention + latency through the ACT th_eff chain).
                    qn = qp.tile([128, 256], bf16, tag="q")
                    nc.vector.tensor_tensor(out=qn[:, :], in0=s[:, :],
                                            in1=s_prev[:, :], op=Alu.add)
                    # TH: th_eff' = BIG*q' + th  (ACT engine, uniform path)
                    th_eff_n = tp.tile([128, 256], fp32, tag="te")
                    if th_uniform is not None:
                        nc.scalar.activation(out=th_eff_n[:, :], in_=qn[:, :],
                                             func=Act.Copy,
                                             bias=float(th_uniform),
                                             scale=float(_BIG))
                    else:
                        nc.vector.scalar_tensor_tensor(
                            out=th_eff_n[:, :], in0=qn[:, :],
                            scalar=float(_BIG), in1=th0_t[:, :],
                            op0=Alu.mult, op1=Alu.add)
                    q_cur = qn
                    th_eff = th_eff_n
                    s_prev = s
                v_prev = v_new
                i_prev = i_cur

            if probe:
                # --- timing probes (identifiable by pc; run once, after the
                # scan) ---
                id_t = singles.tile([128, 128], fp32)
                nc.sync.dma_start(out=id_t[:, :], in_=id_dram[:, :])
                # P1: fp32 identity matmul, 256-col (cost of a PE x-add)
                ps_pr = pspr.tile([128, 256], fp32, tag="pp")
                for _ in range(3):
                    nc.tensor.matmul(ps_pr[:, :], id_t[:, :],
                                     xkeep[:, :], start=True, stop=True)
                pr_sb = singles.tile([128, 256], fp32)
                nc.vector.tensor_copy(out=pr_sb[:, :], in_=ps_pr[:, :])
                nc.sync.dma_start(out=pr_dram[:, :], in_=pr_sb[:, :])
                # P2: bf16 matmul same shape
                id_b = singles.tile([128, 128], bf16)
                nc.vector.tensor_copy(out=id_b[:, :], in_=id_t[:, :])
                xb = singles.tile([128, 256], bf16)
                nc.vector.tensor_copy(out=xb[:, :], in_=xkeep[:, :])
                ps_b = pspr.tile([128, 256], fp32, tag="pp")
                for _ in range(2):
                    nc.tensor.matmul(ps_b[:, :], id_b[:, :], xb[:, :],
                                     start=True, stop=True)
                # P3: GpSimd TT add concurrent with DVE STTs (port-conflict
                # probe): no data deps between the two engines' ops
                ga = singles.tile([128, 256], fp32)
                gb = singles.tile([128, 256], fp32)
                nc.vector.tensor_copy(out=ga[:, :], in_=zero_t[:, :])
                nc.vector.tensor_copy(out=gb[:, :], in_=zero_t[:, :])
                da = singles.tile([128, 256], fp32)
                nc.vector.tensor_copy(out=da[:, :], in_=zero_t[:, :])
                for _ in range(3):
                    nc.gpsimd.tensor_tensor(out=ga[:, :], in0=ga[:, :],
                                            in1=gb[:, :], op=Alu.add)
                    nc.vector.scalar_tensor_tensor(
                        out=da[:, :], in0=da[:, :], scalar=1.0,
                        in1=zero_t[:, :], op0=Alu.mult, op1=Alu.add)
                # P4: f32 reduce (dtype effect on reduce)
                cfx = cfp.tile([128, 4], fp32, tag="cfx")
                for _ in range(2):
                    nc.vector.tensor_reduce(
                        out=cfx[:, :], in_=red_ap(da[:, :]),
                        axis=mybir.AxisListType.X, op=Alu.add,
                        opt_input=False, opt_output=False)
                # P5: all-bf16 flat STT (4x-mode probe)
                sb = singles.tile([128, 256], bf16)
                nc.vector.memset(sb[:, :], 0.0)
                for _ in range(2):
                    nc.vector.scalar_tensor_tensor(
                        out=sb[:, :], in0=sb[:, :], scalar=1.0,
                        in1=zero_bf[:, :], op0=Alu.mult, op1=Alu.add)

    nc.compile()
    return nc


def _prep_inputs(current_in, threshold, beta_mem_raw, beta_syn_raw,
                 neighbor_weights, cluster_gain):
    """Host-side param prep + per-core layout transform."""
    f32 = np.float32
    f64 = np.float64
    beta_m = _sigmoid_f32(beta_mem_raw).reshape(())
    beta_s = _sigmoid_f32(beta_syn_raw).reshape(())
    bm1 = f32(1.0) - beta_m
    Wsig = _sigmoid_f32(neighbor_weights)  # (64, 64)

    gain = np.asarray(cluster_gain, dtype=f32)
    # W~[c',c] = Wsig[c,c'] * gain[c] * beta_s * (1-beta_m) / 64 (one rounding)
    colscale = f64(gain) * f64(beta_s) * f64(bm1) / f64(_K)
    wt64 = f64(Wsig.T) * colscale[None, :]
    wblk = np.zeros((128, 128), dtype=f32)
    wblk[0:64, 0:64] = wt64.astype(f32)
    wblk[64:128, 64:128] = wt64.astype(f32)

    th = np.asarray(threshold, dtype=f32)
    uniform_th = float(th.flat[0]) if np.all(th == th.flat[0]) else None
    th_jc = th.reshape(_K, _NC)  # [j, c]
    th_tile = np.ascontiguousarray(
        np.tile(th_jc.T[:, None, :], (2, 4, 1)).reshape(128, 256), dtype=f32)

    x = np.asarray(current_in, dtype=f32) * bm1   # fold (1-beta_m) into x
    per_core_x = []
    for core in range(_NCORES):
        xl = x[:, core * _BLOC:(core + 1) * _BLOC, :]
        xd = xl.reshape(_T, 2, 4, _K, _NC).transpose(0, 1, 4, 2, 3)
        per_core_x.append(np.ascontiguousarray(xd).reshape(_T, 128, 256))

    return (per_core_x, th_tile, wblk, uniform_th,
            float(beta_s), float(beta_m))


def _gather_output(dev_out):
    """(T,128,256) device layout -> (T, 8, 4096) batch-major."""
    a = dev_out.reshape(_T, 2, _NC, 4, _K).transpose(0, 1, 3, 4, 2)
    return np.ascontiguousarray(a).reshape(_T, _BLOC, _D)


def _run(current_in, threshold, beta_mem_raw, beta_syn_raw,
         neighbor_weights, cluster_gain, trace=False, tmpdir=None,
         force_general=False, probe=False):
    from concourse.bass_utils import run_bass_kernel_spmd

    (per_core_x, th_tile, wblk, uniform_th, beta_s, beta_m) = \
        _prep_inputs(current_in, threshold, beta_mem_raw, beta_syn_raw,
                     neighbor_weights, cluster_gain)

    if force_general:
        uniform_th = None
    nc = _build(beta_s, beta_m, uniform_th, probe=probe)
    in_maps = []
    for c in range(_NCORES):
        m = {"x": per_core_x[c], "wblk": wblk}
        if uniform_th is None:
            m["th"] = th_tile
        if probe:
            m["ident"] = np.eye(128, dtype=np.float32)
        in_maps.append(m)

    res = run_bass_kernel_spmd(nc, in_maps, list(range(_NCORES)),
                               trace=trace, tmpdir=tmpdir)

    spikes = np.empty((_T, _B, _D), dtype=np.float32)
    v_trace = np.empty((_T, _B, _D), dtype=np.float32)
    for core in range(_NCORES):
        b0 = core * _BLOC
        spikes[:, b0:b0 + _BLOC, :] = _gather_output(
            np.asarray(res.results[core]["s_out"], dtype=np.float32))
        v_trace[:, b0:b0 + _BLOC, :] = _gather_output(res.results[core]["v_out"])
    return (spikes, v_trace), res


def kernel(current_in, threshold, beta_mem_raw, beta_syn_raw,
           neighbor_weights, cluster_gain):
    (spikes, v_trace), _ = _run(current_in, threshold, beta_mem_raw,
                                beta_syn_raw, neighbor_weights, cluster_gain)
    return spikes, v_trace
